# revision 1
# baseline (speedup 1.0000x reference)
"""Trainium2 Bass kernel for a 6-layer GPT-2-style decoder (D=768, H=12, B=2, S=1024,
V=50257) on 8 NeuronCores.

Sharding: core c = 4*b + r handles batch b (2 groups of 4 cores) and query-token
chunk r (256 tokens). Activations live on-chip feature-major ("transposed",
[d, tok]), so every linear layer is a plain lhsT.T @ rhs with natural weight
layouts and no on-chip transposes. Per layer, the LN1 output aT is AllGather'd
once within the 4-core group (1 collective/layer); every core then computes
K^T and V for the FULL sequence locally (replicated-KV: cheap on the 64% -idle
PE, and it removes the second collective from the critical path). The K bias
is dropped exactly: softmax over keys is invariant to a constant per-key shift.
Attention computes S^T = K^T.T @ Q^T chunk-by-chunk (k on partitions), exp on
the scalar engine, causal mask as a per-core bf16 multiply, and O^T accumulated
with a ones-augmented V whose extra row yields the softmax denominator.
lm_head is vocab-sharded (1/4 of the vocab per core) after a final AllGather of
the normed hidden states; the core's own 256 tokens are computed from local hfT
while that AllGather is in flight.
"""

import numpy as np
import ml_dtypes

import concourse.bacc as bacc
import concourse.bass as bass
import concourse.tile as tile
from concourse import mybir
from concourse.bass import ts
from concourse.bass_utils import run_bass_kernel_spmd

# Model shape (hardcoded per contract)
V, L, D, H, NL = 50257, 1024, 768, 12, 6
HD = D // H            # 64
B, S = 2, 1024
EPS = 1e-5

P = 128
CH = 256               # tokens per core
DC = D // P            # 6 feature chunks
HP = H // 2            # 6 head pairs
KC = S // P            # 8 key chunks of 128
FC = 4 * D             # 3072
FCC = FC // P          # 24
VSH = (V + 3) // 4     # 12565 logical vocab shard
VPAD = 12800           # padded shard width (25 * 512)
VT = VPAD // 512       # 25 vocab tiles
TT = S // P            # 8 token tiles of 128 in lm_head
AGP = 8                # lm_head vt-prefix computed (own tokens) during h-AG

F32 = mybir.dt.float32
BF16 = mybir.dt.bfloat16
ADD = mybir.AluOpType.add
MULT = mybir.AluOpType.mult
AF = mybir.ActivationFunctionType

REPLICA_GROUPS = [[0, 1, 2, 3], [4, 5, 6, 7]]
GELU = AF.Gelu_apprx_tanh
USE_AG = True  # debug: replace AllGather with local copies when False
CUT = None  # debug: stop after stage 'ln1'|'attn'|'proj' and dump
REPEAT = 1  # timing: run the whole body this many times inside one NEFF


def build_nc(nl: int = NL, lm_head: bool = True) -> bacc.Bacc:
    nc = bacc.Bacc(num_devices=8)

    # ---- I/O declarations (per-core tensors; host pre-arranges layouts) ----
    hT0 = nc.dram_tensor("hT0", [P, DC, CH], F32, kind="ExternalInput")
    wq = nc.dram_tensor("wq", [nl, P, DC, 3 * D], BF16, kind="ExternalInput")
    wp = nc.dram_tensor("wp", [nl, HD, H, D], BF16, kind="ExternalInput")
    wf = nc.dram_tensor("wf", [nl, FCC, P, DC, P], BF16, kind="ExternalInput")
    wm = nc.dram_tensor("wm", [nl, DC, P, FCC, P], BF16, kind="ExternalInput")
    # LN params: [l, p, {g1,b1,g2,b2}, dc]
    lnp = nc.dram_tensor("lnp", [nl, P, 4, DC], F32, kind="ExternalInput")
    lnf = nc.dram_tensor("lnf", [P, 2, DC], F32, kind="ExternalInput")
    bq = nc.dram_tensor("bq", [nl, P, 2 * HP], F32, kind="ExternalInput")  # q,k bias
    vb = nc.dram_tensor("vb", [nl, HD, H], F32, kind="ExternalInput")      # v bias
    bp = nc.dram_tensor("bp", [nl, P, DC], F32, kind="ExternalInput")
    bfc = nc.dram_tensor("bfc", [nl, P, FCC], F32, kind="ExternalInput")
    bm = nc.dram_tensor("bm", [nl, P, DC], F32, kind="ExternalInput")
    Mmask = nc.dram_tensor("Mmask", [P, KC, CH], BF16, kind="ExternalInput")
    wteT = nc.dram_tensor("wteT", [VT, P, DC, 512], BF16, kind="ExternalInput")
    out = nc.dram_tensor("out", [P, TT, VT, 512], BF16, kind="ExternalOutput")

    from contextlib import ExitStack
    with ExitStack() as _es:
        tc = _es.enter_context(tile.TileContext(nc))
        pool = lambda *a, **k: _es.enter_context(tc.tile_pool(*a, **k))
        singles = pool(name="singles", bufs=1)
        acts = pool(name="acts", bufs=2)
        xpool = pool(name="xpool", bufs=1)
        gpool = pool(name="gpool", bufs=1)
        wpool = pool(name="wpool", bufs=1)
        wfpool = pool(name="wfpool", bufs=4)
        wmpool = pool(name="wmpool", bufs=2)
        kvpool = pool(name="kvpool", bufs=1)
        ktpool = pool(name="ktpool", bufs=1)
        ppool = pool(name="ppool", bufs=2)
        stats = pool(name="stats", bufs=2)
        opool = pool(name="opool", bufs=2)
        psA = pool(name="psA", bufs=5, space="PSUM")
        psB = pool(name="psB", bufs=3, space="PSUM")
        dram = pool(name="dram", bufs=2, space="DRAM")
        if True:
            # ---- persistent tiles ----
            hT = singles.tile([P, DC, CH], F32)
            ones_b = singles.tile([P, P], BF16)
            nc.vector.memset(ones_b[:], 1.0)
            ones1_f = singles.tile([1, HD], BF16)
            nc.vector.memset(ones1_f[:], 1.0)
            M_sb = singles.tile([P, KC, CH], BF16)
            nc.sync.dma_start(M_sb[:], Mmask[:])
            eps_sb = singles.tile([P, 1], F32)
            nc.vector.memset(eps_sb[:], EPS)

            def layernorm(g_ap, b_ap, out_dtype=BF16):
                """LN over feature dim (partitions) of hT via ones-matmul
                broadcast-stats. Returns bf16 [P, DC, CH] tile."""
                xb = xpool.tile([P, DC, CH], BF16, tag="ln_xb")
                nc.vector.tensor_copy(xb[:].rearrange("p a b -> p (a b)"),
                                      hT[:].rearrange("p a b -> p (a b)"))
                xsq = xpool.tile([P, DC, CH], BF16, tag="ln_xsq")
                nc.scalar.square(xsq[:].rearrange("p a b -> p (a b)"),
                                 hT[:].rearrange("p a b -> p (a b)"))
                ps1 = psB.tile([P, CH], F32, tag="pss")
                ps2 = psB.tile([P, CH], F32, tag="pss")
                for dc in range(DC):
                    nc.tensor.matmul(ps1[:], ones_b[:], xb[:, dc, :],
                                     start=dc == 0, stop=dc == DC - 1)
                for dc in range(DC):
                    nc.tensor.matmul(ps2[:], ones_b[:], xsq[:, dc, :],
                                     start=dc == 0, stop=dc == DC - 1)
                mean = stats.tile([P, CH], F32, tag="ln_mean")
                nc.vector.tensor_scalar_mul(mean[:], ps1[:], 1.0 / D)
                var = stats.tile([P, CH], F32, tag="ln_var")
                nc.vector.tensor_scalar_mul(var[:], ps2[:], 1.0 / D)
                msq = stats.tile([P, CH], F32, tag="ln_msq")
                nc.vector.tensor_mul(msq[:], mean[:], mean[:])
                nc.vector.tensor_sub(var[:], var[:], msq[:])
                sd = stats.tile([P, CH], F32, tag="ln_sd")
                nc.scalar.activation(sd[:], var[:], AF.Sqrt, bias=eps_sb[:])
                rstd = stats.tile([P, CH], F32, tag="ln_rstd")
                nc.vector.reciprocal_approx_fast(rstd[:], sd[:])
                o = acts.tile([P, DC, CH], out_dtype, tag="ln_out")
                for dc in range(DC):
                    t = stats.tile([P, CH], F32, tag="ln_tmp")
                    nc.vector.tensor_sub(t[:], hT[:, dc, :], mean[:])
                    nc.vector.tensor_mul(t[:], t[:], rstd[:])
                    nc.vector.tensor_scalar(o[:, dc, :], t[:],
                                            g_ap[:, dc:dc + 1], b_ap[:, dc:dc + 1],
                                            op0=MULT, op1=ADD)
                return o

            def dump_and_out(ap_2d, width):
                """DMA a [p, width] view into out[:, :, 0, :] slabs (debug)."""
                n_slab = (width + 511) // 512
                dbgt = opool.tile([ap_2d.shape[0], width], BF16, tag="dbg_cut")
                nc.vector.tensor_copy(dbgt[:], ap_2d)
                nc.sync.dma_start(
                    out[0:ap_2d.shape[0], 0:n_slab, 0, :],
                    dbgt[:].rearrange("p (c d) -> p c d", d=512))

            for rep in range(REPEAT):
                nc.sync.dma_start(hT[:], hT0[:])
                for l in range(nl):
                    # per-layer params to SBUF
                    ln_sb = acts.tile([P, 4, DC], F32, tag="ln_params")
                    nc.sync.dma_start(ln_sb[:], lnp[l])
                    bq_sb = acts.tile([P, 2 * HP], F32, tag="bq_sb")
                    nc.sync.dma_start(bq_sb[:], bq[l])
                    vb_sb = acts.tile([HD, H], F32, tag="vb_sb")
                    nc.sync.dma_start(vb_sb[:], vb[l])
                    bp_sb = acts.tile([P, DC], F32, tag="bp_sb")
                    nc.sync.dma_start(bp_sb[:], bp[l])
                    bfc_sb = acts.tile([P, FCC], F32, tag="bfc_sb")
                    nc.sync.dma_start(bfc_sb[:], bfc[l])
                    bm_sb = acts.tile([P, DC], F32, tag="bm_sb")
                    nc.sync.dma_start(bm_sb[:], bm[l])

                    # ---------- attention ----------
                    aT = layernorm(ln_sb[:, 0, :], ln_sb[:, 1, :])

                    if CUT == "ln1":
                        dump_and_out(aT[:].rearrange("p a b -> p (a b)"), DC * CH)
                        break

                    # AllGather aT across the 4-core group as early as possible;
                    # Q (local) overlaps the collective.
                    a_in = dram.tile([D, CH], BF16, tag="a_in")
                    a_out = dram.tile([4, D, CH], BF16, tag="a_out")
                    nc.sync.dma_start(
                        a_in[:].rearrange("(dc p) c -> p dc c", p=P), aT[:])
                    if USE_AG:
                        nc.gpsimd.collective_compute(
                            "AllGather", mybir.AluOpType.bypass,
                            replica_groups=REPLICA_GROUPS,
                            ins=[a_in[:].opt()], outs=[a_out[:].opt()])
                    else:
                        for _r in range(4):
                            nc.gpsimd.dma_start(a_out[_r], a_in[:])

                    wq_sb = wpool.tile([P, DC, 3 * D], BF16, tag="wq_sb")
                    nc.sync.dma_start(wq_sb[:], wq[l])

                    QT = acts.tile([P, HP, CH], BF16, tag="QT")
                    for hp in range(HP):
                        psq = psA.tile([P, 512], F32, tag="ps", name="psq")[:, :CH]
                        for dc in range(DC):
                            nc.tensor.matmul(psq[:], wq_sb[:, dc, ts(hp, P)],
                                             aT[:, dc, :],
                                             start=dc == 0, stop=dc == DC - 1)
                        nc.vector.tensor_scalar_add(QT[:, hp, :], psq[:],
                                                    bq_sb[:, hp:hp + 1])

                    # gathered activations back to SBUF, per source rank so the
                    # replicated K/V compute can start on rank 0's chunk early
                    aA = kvpool.tile([P, 4, DC, CH], BF16, tag="aA")
                    for rr in range(4):
                        nc.sync.dma_start(
                            aA[:, rr],
                            a_out[rr].rearrange("(dc p) c -> p dc c", p=P))

                    # K^T for the full sequence, feature-major (no K bias:
                    # softmax is invariant to a constant per-key shift).
                    # Two source ranks share one [P,512] PSUM tile per copy.
                    KA = ktpool.tile([P, HP, S], BF16, tag="KA")
                    for rpair in range(2):
                        for hp in range(HP):
                            psk = psA.tile([P, 512], F32, tag="ps", name="psk")
                            for ri in range(2):
                                rr = 2 * rpair + ri
                                for dc in range(DC):
                                    nc.tensor.matmul(
                                        psk[:, ri * CH:(ri + 1) * CH],
                                        wq_sb[:, dc, D + hp * P:D + (hp + 1) * P],
                                        aA[:, rr, dc, :],
                                        start=dc == 0, stop=dc == DC - 1)
                            dst = KA[:, hp, rpair * 512:(rpair + 1) * 512]
                            if hp % 2 == 0:
                                nc.vector.tensor_copy(dst, psk[:])
                            else:
                                nc.scalar.activation(dst, psk[:], AF.Copy)

                    if CUT == "q":
                        dump_and_out(QT[:].rearrange("p a b -> p (a b)"), HP * CH)
                        break
                    if CUT == "ka":
                        dump_and_out(KA[:, 0, :], S)
                        break
                    # V for the full sequence, token-major with a ones column
                    # (row HD) that accumulates the softmax denominator
                    Vsb = kvpool.tile([P, KC, H, HD + 1], BF16, tag="Vsb")
                    nc.vector.memset(Vsb[:, :, :, HD:HD + 1], 1.0)
                    for rr in range(4):
                        for tc_i in range(2):
                            for nn in range(2):
                                psv = psA.tile([P, 512], F32, tag="ps",
                                               name="psv")[:, :384]
                                for dc in range(DC):
                                    nc.tensor.matmul(
                                        psv[:], aA[:, rr, dc, ts(tc_i, P)],
                                        wq_sb[:, dc, 2 * D + nn * 384:2 * D + (nn + 1) * 384],
                                        start=dc == 0, stop=dc == DC - 1)
                                vdst = Vsb[:, 2 * rr + tc_i, 6 * nn:6 * (nn + 1), 0:HD]
                                vsrc = psv[:].rearrange("p (h f) -> p h f", f=HD)
                                nc.vector.tensor_copy(vdst, vsrc)

                    if CUT == "vsb":
                        vdump = opool.tile([P, KC, HD], BF16, tag="vdump")
                        nc.vector.tensor_copy(vdump[:], Vsb[:, :, 0, 0:HD])
                        dump_and_out(vdump[:].rearrange("p a b -> p (a b)"),
                                     KC * HD)
                        break
                    attnT = acts.tile([HD, H, CH], BF16, tag="attnT")
                    for hp in range(HP):
                        PTs = [ppool.tile([P, KC, CH], BF16, tag="PT",
                                          name=f"PT{par}") for par in range(2)]
                        # interleave even/odd head score MMs: disjoint PE row
                        # groups (base partition 0 / 64) can run concurrently
                        for kc2 in range(KC // 2):
                            pss = [psA.tile([P, 512], F32, tag="ps",
                                            name=f"ps_s{par}") for par in range(2)]
                            for half, par in [(0, 0), (0, 1), (1, 0), (1, 1)]:
                                kc = 2 * kc2 + half
                                hlo = par * HD
                                nc.tensor.matmul(
                                    pss[par][:, half * CH:(half + 1) * CH],
                                    KA[hlo:hlo + HD, hp, ts(kc, P)],
                                    QT[hlo:hlo + HD, hp, :],
                                    start=True, stop=True)
                            for par in range(2):
                                nc.scalar.activation(
                                    PTs[par][:, 2 * kc2:2 * kc2 + 2, :]
                                    .rearrange("p a b -> p (a b)"),
                                    pss[par][:], AF.Exp, scale=1.0 / np.sqrt(HD))
                        if CUT == "pt":
                            dump_and_out(PTs[0][:].rearrange("p a b -> p (a b)"),
                                         KC * CH)
                            break
                        for par in range(2):
                            h = 2 * hp + par
                            PT = PTs[par]
                            nc.vector.tensor_mul(
                                PT[:].rearrange("p a b -> p (a b)"),
                                PT[:].rearrange("p a b -> p (a b)"),
                                M_sb[:].rearrange("p a b -> p (a b)"))
                            ps_o = psB.tile([P, CH], F32, tag="pss")
                            for kc in range(KC):
                                nc.tensor.matmul(ps_o[0:HD + 1, :],
                                                 Vsb[:, kc, h, :], PT[:, kc, :],
                                                 start=kc == 0, stop=kc == KC - 1)
                            den = stats.tile([1, CH], F32, tag="den")
                            nc.vector.tensor_copy(den[:], ps_o[HD:HD + 1, :])
                            rdf = stats.tile([1, CH], F32, tag="rdf")
                            nc.vector.reciprocal_approx_fast(rdf[:], den[:])
                            rd = stats.tile([1, CH], BF16, tag="rd")
                            nc.vector.tensor_copy(rd[:], rdf[:])
                            ps_b = psB.tile([P, CH], F32, tag="pss")
                            nc.tensor.matmul(ps_b[0:HD, :], ones1_f[:], rd[:],
                                             start=True, stop=True)
                            rb_sb = stats.tile([HD, CH], F32, tag="rb_sb")
                            nc.vector.tensor_copy(rb_sb[:], ps_b[0:HD, :])
                            onorm = stats.tile([HD, CH], F32, tag="onorm")
                            nc.vector.tensor_mul(onorm[:], ps_o[0:HD, :], rb_sb[:])
                            nc.vector.tensor_scalar_add(attnT[:, h, :], onorm[:],
                                                        vb_sb[0:HD, h:h + 1])

                    if CUT == "pt":
                        break
                    if CUT == "attn":
                        dump_and_out(attnT[:].rearrange("p a b -> p (a b)"), H * CH)
                        break
                    # ---------- attn proj + residual (per-head contraction) ----------
                    wp_sb = wpool.tile([HD, H, D], BF16, tag="wp_sb")
                    nc.sync.dma_start(wp_sb[:], wp[l])
                    for oc in range(DC):
                        psp = psA.tile([P, 512], F32, tag="ps", name="psp")[:, :CH]
                        for h in range(H):
                            nc.tensor.matmul(psp[:], wp_sb[:, h, ts(oc, P)],
                                             attnT[:, h, :],
                                             start=h == 0, stop=h == H - 1)
                        t = stats.tile([P, CH], F32, tag="res_tmp")
                        nc.vector.tensor_scalar_add(t[:], psp[:], bp_sb[:, oc:oc + 1])
                        nc.vector.tensor_add(hT[:, oc, :], hT[:, oc, :], t[:])

                    if CUT == "proj":
                        dump_and_out(hT[:].rearrange("p a b -> p (a b)"), DC * CH)
                        break
                    # ---------- MLP ----------
                    mT = layernorm(ln_sb[:, 2, :], ln_sb[:, 3, :])
                    gT = gpool.tile([P, FCC, CH], BF16, tag="gT")
                    for oc in range(FCC):
                        wf_sb = wfpool.tile([P, DC, P], BF16, tag="wf_sb")
                        nc.sync.dma_start(wf_sb[:], wf[l, oc])
                        psf = psA.tile([P, 512], F32, tag="ps", name="psf")[:, :CH]
                        for dc in range(DC):
                            nc.tensor.matmul(psf[:], wf_sb[:, dc, :], mT[:, dc, :],
                                             start=dc == 0, stop=dc == DC - 1)
                        nc.scalar.activation(gT[:, oc, :], psf[:], GELU,
                                             bias=bfc_sb[:, oc:oc + 1])
                    for oc in range(DC):
                        wm_sb = wmpool.tile([P, FCC, P], BF16, tag="wm_sb")
                        nc.sync.dma_start(wm_sb[:], wm[l, oc])
                        psm = psA.tile([P, 512], F32, tag="ps", name="psm")[:, :CH]
                        for hc in range(FCC):
                            nc.tensor.matmul(psm[:], wm_sb[:, hc, :], gT[:, hc, :],
                                             start=hc == 0, stop=hc == FCC - 1)
                        t = stats.tile([P, CH], F32, tag="res_tmp")
                        nc.vector.tensor_scalar_add(t[:], psm[:], bm_sb[:, oc:oc + 1])
                        nc.vector.tensor_add(hT[:, oc, :], hT[:, oc, :], t[:])

                # ---------- final LN + h AllGather + lm_head ----------
                if CUT is not None:
                    continue
                lnf_sb = acts.tile([P, 2, DC], F32, tag="lnf_sb")
                nc.sync.dma_start(lnf_sb[:], lnf[:])
                hfT = layernorm(lnf_sb[:, 0, :], lnf_sb[:, 1, :])

                if lm_head:
                    h_in = dram.tile([D, CH], BF16, tag="h_in")
                    h_out = dram.tile([4, D, CH], BF16, tag="h_out")
                    nc.sync.dma_start(
                        h_in[:].rearrange("(dc p) c -> p dc c", p=P), hfT[:])
                    if USE_AG:
                        nc.gpsimd.collective_compute(
                            "AllGather", mybir.AluOpType.bypass,
                            replica_groups=REPLICA_GROUPS,
                            ins=[h_in[:].opt()], outs=[h_out[:].opt()])
                    else:
                        for _r in range(4):
                            nc.gpsimd.dma_start(h_out[_r], h_in[:])
                    # reuses the aA allocation (same shape/tag, never live at
                    # the same time as the per-layer gathers)
                    hAll = kvpool.tile([P, 4, DC, CH], BF16, tag="aA")
                    nc.sync.dma_start(
                        hAll[:],
                        h_out[:].rearrange("r (dc p) c -> p r dc c", p=P))

                    for vt in range(VT):
                        wte_sb = wmpool.tile([P, DC, 512], BF16, tag="wte_sb")
                        nc.sync.dma_start(wte_sb[:], wteT[vt])
                        for tp in range(TT // 2):
                            ot = opool.tile([P, 2, 512], BF16, tag="out_sb")
                            for half in range(2):
                                tt = 2 * tp + half
                                r, tc_i = tt // 2, tt % 2
                                pso = psA.tile([P, 512], F32, tag="ps")
                                for dc in range(DC):
                                    nc.tensor.matmul(
                                        pso[:], hAll[:, r, dc, ts(tc_i, P)],
                                        wte_sb[:, dc, :],
                                        start=dc == 0, stop=dc == DC - 1)
                                # PSUM drain split across scalar/vector engines
                                if half == 0:
                                    nc.scalar.activation(ot[:, half, :], pso[:],
                                                         AF.Copy)
                                else:
                                    nc.vector.tensor_copy(ot[:, half, :], pso[:])
                            nc.sync.dma_start(out[:, 2 * tp:2 * tp + 2, vt, :],
                                              ot[:])
                else:
                    # debug mode: dump hfT into the first slab of out
                    dbg = opool.tile([P, DC, CH], BF16, tag="dbg")
                    nc.vector.tensor_copy(dbg[:].rearrange("p a b -> p (a b)"),
                                          hfT[:].rearrange("p a b -> p (a b)"))
                    nc.sync.dma_start(
                        out[:, 0:3, 0, :],
                        dbg[:].rearrange("p a b -> p (a b)").rearrange(
                            "p (a b) -> p a b", b=512))



    nc.compile()
    return nc


# ----------------------------------------------------------------------------
# Host-side sharding / input prep
# ----------------------------------------------------------------------------

def _bf16(a):
    return np.ascontiguousarray(a.astype(ml_dtypes.bfloat16))


def _prep_shared(inputs, nl):
    """Weights/params identical on every core (pre-arranged layouts)."""
    qkv_w = np.asarray(inputs["qkv_w"], np.float32)[:nl]
    attn_proj_w = np.asarray(inputs["attn_proj_w"], np.float32)[:nl]
    fc_w = np.asarray(inputs["fc_w"], np.float32)[:nl]
    mlp_proj_w = np.asarray(inputs["mlp_proj_w"], np.float32)[:nl]
    d = {}
    d["wq"] = _bf16(qkv_w.reshape(nl, DC, P, 3 * D).transpose(0, 2, 1, 3))
    d["wp"] = _bf16(attn_proj_w.reshape(nl, H, HD, D).transpose(0, 2, 1, 3))
    # wf: [l, oc, p, dc, j] = fc_w[l, 128dc+p, 128oc+j]
    wf4 = fc_w.reshape(nl, DC, P, FCC, P)
    d["wf"] = _bf16(wf4.transpose(0, 3, 2, 1, 4))
    # wm: [l, oc, p, hc, j] = mlp_proj_w[l, 128hc+p, 128oc+j]
    wm4 = mlp_proj_w.reshape(nl, FCC, P, DC, P)
    d["wm"] = _bf16(wm4.transpose(0, 3, 2, 1, 4))

    def pp(v):  # [nl, X*P] -> [nl, P, X]
        x = v.shape[-1] // P
        return np.ascontiguousarray(
            v.reshape(v.shape[0], x, P).transpose(0, 2, 1).astype(np.float32))

    g1 = np.asarray(inputs["ln1_g"], np.float32)[:nl]
    b1 = np.asarray(inputs["ln1_b"], np.float32)[:nl]
    g2 = np.asarray(inputs["ln2_g"], np.float32)[:nl]
    b2 = np.asarray(inputs["ln2_b"], np.float32)[:nl]
    d["lnp"] = np.ascontiguousarray(
        np.stack([pp(g1), pp(b1), pp(g2), pp(b2)], axis=2))  # [nl,P,4,DC]
    gf = np.asarray(inputs["lnf_g"], np.float32)[None]
    bf_ = np.asarray(inputs["lnf_b"], np.float32)[None]
    d["lnf"] = np.ascontiguousarray(np.stack([pp(gf)[0], pp(bf_)[0]], axis=1))
    qkv_b = np.asarray(inputs["qkv_b"], np.float32)[:nl]
    d["bq"] = pp(qkv_b[:, 0:2 * D])
    d["vb"] = np.ascontiguousarray(qkv_b[:, 2 * D:3 * D].reshape(nl, H, HD).transpose(0, 2, 1))
    d["bp"] = pp(np.asarray(inputs["attn_proj_b"], np.float32)[:nl])
    d["bfc"] = pp(np.asarray(inputs["fc_b"], np.float32)[:nl])
    d["bm"] = pp(np.asarray(inputs["mlp_proj_b"], np.float32)[:nl])
    return d


def _prep_percore(inputs, core, wteT_shards):
    b, r = core // 4, core % 4
    x = np.asarray(inputs["x"])
    wte = np.asarray(inputs["wte"], np.float32)
    wpe = np.asarray(inputs["wpe"], np.float32)
    h0 = wte[x[b]] + wpe[:S]                      # [S, D] f32
    chunk = h0[r * CH:(r + 1) * CH]               # [CH, D]
    hT0 = np.ascontiguousarray(
        chunk.T.reshape(DC, P, CH).transpose(1, 0, 2))  # [P, DC, CH]
    # causal mask: M[p, kc, j] = 1 if 128*kc + p <= 256*r + j
    k_idx = (np.arange(KC)[:, None] * P + np.arange(P)[None, :])  # [kc, p]
    q_idx = r * CH + np.arange(CH)
    M = (k_idx[:, :, None] <= q_idx[None, None, :])  # [kc, p, j]
    M = np.ascontiguousarray(M.transpose(1, 0, 2).astype(ml_dtypes.bfloat16))
    return {"hT0": hT0, "Mmask": M, "wteT": wteT_shards[r]}


def _make_wteT_shards(inputs):
    wte = np.asarray(inputs["wte"], np.float32)
    shards = []
    for r in range(4):
        lo = r * VSH
        hi = min(lo + VSH, V)
        w = np.zeros((VPAD, D), np.float32)
        w[:hi - lo] = wte[lo:hi]
        # [vt, p, dc, c] = w[512*vt + c, 128*dc + p]
        t = w.reshape(VT, 512, DC, P).transpose(0, 3, 2, 1)
        shards.append(_bf16(t))
    return shards


_NC_CACHE = {}


def _get_nc(nl=NL, lm_head=True):
    key = (nl, lm_head)
    if key not in _NC_CACHE:
        _NC_CACHE[key] = build_nc(nl, lm_head)
    return _NC_CACHE[key]


def run_cores(inputs, nl=NL, lm_head=True, **run_kwargs):
    nc = _get_nc(nl, lm_head)
    shared = _prep_shared(inputs, nl)
    wteT_shards = _make_wteT_shards(inputs)
    in_maps = []
    for core in range(8):
        m = dict(shared)
        m.update(_prep_percore(inputs, core, wteT_shards))
        in_maps.append(m)
    return run_bass_kernel_spmd(nc, in_maps, core_ids=list(range(8)), **run_kwargs)


def kernel(**inputs) -> np.ndarray:
    res = run_cores(inputs)
    logits = np.empty((B, S, V), np.float32)
    for core in range(8):
        b, r = core // 4, core % 4
        o = np.asarray(res.results[core]["out"], dtype=np.float32)  # [P,TT,VT,512]
        shard = o.transpose(1, 0, 2, 3).reshape(S, VPAD)
        lo = r * VSH
        hi = min(lo + VSH, V)
        logits[b, :, lo:hi] = shard[:, :hi - lo]
    return logits

